# revision 78
# baseline (speedup 1.0000x reference)
"""Bass/Trainium2 kernel v4 for 2-layer GAT (nn_GAT_48919677501958).

Contract: kernel(**inputs) takes FULL unsharded numpy inputs, returns the
FULL [10000, 40] float32 output.

v4 strategy: 3-block split-table pipelining + fp8 one-hots.
  - Nodes are dealt to 8 cores x 10 tiles of 128 dsts.  The shared layer-1
    table `haug` has 2 blocks (a = every core's tiles [0,6), b = tiles
    [6,10)); the layer-2 table `h2tab` has 3 blocks (a = tiles [0,6),
    b1 = [6,8), b2 = [8,10)).  Each block is AllGather'ed separately into
    slices of one tensor, so
      * B-phase gathers of early-src edges start right after A finishes its
        first 6 tiles,
      * D-phase gathers of a-srcs run while the B pipeline drains, b1-srcs
        while the last two b_backs run, and only the b2 srcs (~20% of
        edges) wait for the final tile.
  - Per (core, tile) the edge list is split into aligned a/b1/b2 ranges
    (SPMD-uniform group counts; spill goes to later ranges, padding
    minimized by search).  B gathers 3 parts per tile; D gathers 2 big
    chunks per block.
  - ct2 (the [dst, edge] one-hot for PE broadcasts) is fp8 (exact 0/1).
  - exp(leaky_relu) is computed as Prelu(alpha) -> Exp on ACT (same HW
    activation-function set, no table reload).
  - haug rows are written as 520 used elems (no pad write); constants are
    packed into one DRAM blob; chain ops are issued batched per engine.
"""

from dataclasses import dataclass, field

import numpy as np

import concourse.bass as bass
import concourse.mybir as mybir
import concourse.tile as tile
from concourse.bass_utils import run_bass_kernel_spmd
from concourse.masks import make_identity

F32 = mybir.dt.float32
BF16 = mybir.dt.bfloat16
FP8 = mybir.dt.float8e4
I16 = mybir.dt.int16

NEG_SLOPE = 0.2
P = 128


@dataclass
class Cfg:
    n_nodes: int = 10000
    n_cores: int = 8
    tpc: int = 10
    ka: int = 6   # tiles [0,ka) form block-a
    kb1: int = 6  # tiles [ka,kb1) form block-b1; rest b2
    d_in: int = 256
    hid: int = 64
    heads: int = 8
    d_out: int = 40
    ga: list[int] = field(default_factory=list)
    gb1: list[int] = field(default_factory=list)
    gb2: list[int] = field(default_factory=list)
    collective: bool = True
    phases: str = "ABCD"

    @property
    def npc(self):
        return self.tpc * P

    @property
    def npad(self):
        return self.n_cores * self.npc

    @property
    def na_rows(self):  # rows in block-a
        return self.n_cores * self.ka * P

    @property
    def nb1_rows(self):
        return self.n_cores * (self.kb1 - self.ka) * P

    @property
    def d_hid(self):
        return self.hid * self.heads

    @property
    def rw1(self):  # table-1 row elems: 512 h + 8 alpha_src + pad (256B rule)
        return 640

    @property
    def rw1u(self):
        return self.d_hid + self.heads

    @property
    def rw2(self):
        return 128

    @property
    def sum_ng(self):
        return sum(self.ga) + sum(self.gb1) + sum(self.gb2)

    @property
    def ng_max(self):
        return max(max(self.ga, default=1), max(self.gb1, default=1),
                   max(self.gb2, default=1), 1)

    # global column order: [t0a..t9a | t0b1..t9b1 | t0b2..t9b2]
    def offs_a(self, t):
        return sum(self.ga[:t])

    def offs_b1(self, t):
        return sum(self.ga) + sum(self.gb1[:t])

    def offs_b2(self, t):
        return sum(self.ga) + sum(self.gb1) + sum(self.gb2[:t])


def _wrap_idx(flat: np.ndarray) -> np.ndarray:
    assert flat.size % 16 == 0
    w = np.ascontiguousarray(flat.reshape(-1, 16).T).astype(np.int16)
    return np.tile(w, (8, 1))


def _split_groups(cnt_first, cnt_rest):
    """groups for the first range (tie-max pad-min), spilling into rest."""
    P_ = P
    best = None
    for g in range(0, int(cnt_first.max()) // P_ + 1):
        placed = np.minimum(cnt_first, g * P_)
        rest = cnt_first - placed + cnt_rest
        grest = int(np.ceil(rest.max() / P_)) if rest.max() else 0
        pad = (g * P_ - placed).sum() + (grest * P_ - rest).sum()
        if best is None or pad <= best[0]:
            best = (pad, g, grest)
    return best[1]


def preprocess(cfg: Cfg, x, edge_index, W1, att_src1, att_dst1, b1, W2,
               att_src2, att_dst2, b2):
    import ml_dtypes
    N = cfg.n_nodes
    KA, KB1 = cfg.ka, cfg.kb1
    NA = cfg.na_rows
    src = np.concatenate([np.asarray(edge_index[0], np.int64), np.arange(N)])
    dst = np.concatenate([np.asarray(edge_index[1], np.int64), np.arange(N)])
    deg = np.bincount(dst, minlength=N)

    order_e = np.argsort(dst, kind="stable")
    sorted_src = src[order_e]
    starts = np.zeros(N + 1, np.int64)
    np.cumsum(deg, out=starts[1:])

    node_order = np.argsort(-deg, kind="stable")
    n_tiles = cfg.n_cores * cfg.tpc
    tiles = np.full((n_tiles, P), -1, np.int64)
    wts = np.array([0.42] + [1.13] * (cfg.tpc - 3) + [0.82, 0.42])

    for c in range(cfg.n_cores):
        mine = node_order[c::cfg.n_cores]
        tgt = wts / wts.sum() * deg[mine].sum()
        cur = np.zeros(cfg.tpc)
        cnt = np.zeros(cfg.tpc, np.int64)
        cap = 18 * P - 16
        for n in mine:
            free = cnt < P
            ok = free & (cur + deg[n] <= cap)
            pick = ok if ok.any() else free
            t = int(np.argmax(np.where(pick, tgt - cur, -np.inf)))
            tiles[c * cfg.tpc + t, cnt[t]] = n
            cur[t] += deg[n]
            cnt[t] += 1

    node_of_slot = np.full((cfg.n_cores, cfg.npc), -1, np.int64)
    for c in range(cfg.n_cores):
        for t in range(cfg.tpc):
            node_of_slot[c, t * P:(t + 1) * P] = tiles[c * cfg.tpc + t]
    flat_slots = node_of_slot.reshape(-1)
    real = flat_slots >= 0
    slot_of_node = np.full(N, -1, np.int64)
    slot_of_node[flat_slots[real]] = np.nonzero(real)[0]
    assert (slot_of_node >= 0).all()

    # table rows per slot for both layers
    slot_arr = np.arange(cfg.npad)
    c_arr, r_arr = slot_arr // cfg.npc, slot_arr % cfg.npc
    t_arr, p_arr = r_arr // P, r_arr % P
    # layer-1 (haug): a-block + one b-block (c-major over tiles [KA,10))
    trow1 = np.where(
        t_arr < KA, c_arr * (KA * P) + t_arr * P + p_arr,
        NA + c_arr * ((cfg.tpc - KA) * P) + (t_arr - KA) * P + p_arr)
    # layer-2 (h2tab): a, b1, b2 blocks
    nb1t = KB1 - KA
    nb2t = cfg.tpc - KB1
    trow2 = np.where(
        t_arr < KA, c_arr * (KA * P) + t_arr * P + p_arr,
        np.where(
            t_arr < KB1,
            c_arr * (nb1t * P) + (t_arr - KA) * P + p_arr,  # local to h2tb1
            NA + c_arr * (nb2t * P) + (t_arr - KB1) * P + p_arr))
    rng_of_slot = np.where(t_arr < KA, 0, np.where(t_arr < KB1, 1, 2))

    # per (core, tile): edges split into 3 src-ranges, dst-major
    ed = [[None] * cfg.tpc for _ in range(cfg.n_cores)]
    for c in range(cfg.n_cores):
        for t in range(cfg.tpc):
            parts = [[[], [], []] for _ in range(3)]  # rng -> [r1, r2, dloc]
            for d in range(P):
                n = node_of_slot[c, t * P + d]
                if n < 0:
                    continue
                ss = sorted_src[starts[n]:starts[n] + deg[n]]
                sl = slot_of_node[ss]
                rg = rng_of_slot[sl]
                for q in range(3):
                    m = rg == q
                    parts[q][0].append(trow1[sl[m]])
                    parts[q][1].append(trow2[sl[m]])
                    parts[q][2].append(np.full(int(m.sum()), d, np.int64))
            ed[c][t] = [tuple(np.concatenate(pp) if pp else
                              np.zeros(0, np.int64) for pp in parts[q])
                        for q in range(3)]

    # choose group counts: a (spill->b1), b1 (spill->b2), b2 = ceil rest
    cfg.ga, cfg.gb1, cfg.gb2 = [], [], []
    for t in range(cfg.tpc):
        n0 = np.array([ed[c][t][0][0].size for c in range(cfg.n_cores)])
        n1 = np.array([ed[c][t][1][0].size for c in range(cfg.n_cores)])
        n2 = np.array([ed[c][t][2][0].size for c in range(cfg.n_cores)])
        ga = _split_groups(n0, n1 + n2)
        sp0 = n0 - np.minimum(n0, ga * P)
        if KB1 > KA:
            gb1 = int(np.ceil((sp0 + n1).max() / P)) if (sp0 + n1).max() \
                else 0
            gb2 = int(np.ceil(n2.max() / P)) if n2.max() else 0
        else:
            gb1 = 0
            gb2 = int(np.ceil((sp0 + n2).max() / P)) if (sp0 + n2).max() \
                else 0
        cfg.ga.append(ga)
        cfg.gb1.append(gb1)
        cfg.gb2.append(gb2)

    # weights / constants
    ablk_s = np.zeros((cfg.d_hid, cfg.heads), np.float32)
    ablk_d = np.zeros((cfg.d_hid, cfg.heads), np.float32)
    a_s1 = np.asarray(att_src1, np.float32)
    a_d1 = np.asarray(att_dst1, np.float32)
    for h in range(cfg.heads):
        ablk_s[h * cfg.hid:(h + 1) * cfg.hid, h] = a_s1[h]
        ablk_d[h * cfg.hid:(h + 1) * cfg.hid, h] = a_d1[h]
    W1 = np.asarray(W1, np.float32)
    Wa1 = np.concatenate([W1 @ ablk_s, W1 @ ablk_d], axis=1)
    W2 = np.asarray(W2, np.float32)
    w2s = W2 @ np.asarray(att_src2, np.float32)[0]
    w2d = W2 @ np.asarray(att_dst2, np.float32)[0]
    W2a = np.concatenate([W2, w2s[:, None], w2d[:, None]], axis=1)
    b1r = np.tile(np.asarray(b1, np.float32)[None, :], (P, 1))
    b2r = np.tile(np.asarray(b2, np.float32)[None, :], (P, 1))
    j = np.arange(cfg.d_hid)
    old = (j % cfg.heads) * cfg.hid + j // cfg.heads
    b1r = np.ascontiguousarray(b1r[:, old])
    W2a = np.ascontiguousarray(W2a[old, :])

    NGM = cfg.ng_max
    iotaT = np.repeat(np.arange(P, dtype=np.float32), NGM)
    iotaT = np.tile(iotaT[None, :], (P, 1))

    xT = np.zeros((cfg.d_in, cfg.npad), np.float32)
    xT[:, np.nonzero(real)[0]] = np.asarray(x, np.float32).T[:, flat_slots[real]]

    KT = cfg.d_in // P
    K2 = cfg.d_hid // P
    in_maps = []
    for c in range(cfg.n_cores):
        g1_parts, g2_parts, di_parts = [], [], []
        for q, gl in ((0, cfg.ga), (1, cfg.gb1), (2, cfg.gb2)):
            for t in range(cfg.tpc):
                ns = gl[t] * P
                r1 = np.zeros(ns, np.int64)
                r2 = np.zeros(ns, np.int64)
                dl = np.full(ns, 255, np.int64)
                e0, e1, e2 = ed[c][t]
                na = cfg.ga[t] * P
                nb = cfg.gb1[t] * P
                sp0 = max(0, e0[0].size - na)  # a-spill count
                if q == 0:
                    pools = [(e0, 0, min(e0[0].size, na))]
                elif q == 1:
                    pools = ([(e0, e0[0].size - sp0, e0[0].size),
                              (e1, 0, e1[0].size)] if KB1 > KA else [])
                elif KB1 > KA:
                    pools = [(e2, 0, e2[0].size)]
                else:
                    pools = [(e0, e0[0].size - sp0, e0[0].size),
                             (e2, 0, e2[0].size)]
                z = [np.zeros(0, np.int64)]
                rr1 = np.concatenate([p[0][lo:hi] for p, lo, hi in pools]
                                     or z)
                rr2 = np.concatenate([p[1][lo:hi] for p, lo, hi in pools]
                                     or z)
                rrd = np.concatenate([p[2][lo:hi] for p, lo, hi in pools]
                                     or z)
                assert rr1.size <= ns, (q, t, rr1.size, ns)
                r1[:rr1.size] = rr1
                r2[:rr2.size] = rr2
                dl[:rrd.size] = rrd
                g1_parts.append(r1)
                g2_parts.append(r2)
                di_parts.append(dl.reshape(gl[t], P).T if gl[t] else
                                np.zeros((P, 0), np.int64))
        gi = _wrap_idx(np.concatenate(g1_parts))
        gi2 = _wrap_idx(np.concatenate(g2_parts))
        dstidx = np.concatenate(di_parts, axis=1).astype(np.float32)
        ct2 = (dstidx[None, :, :] == np.arange(P)[:, None, None])
        ct2 = np.ascontiguousarray(
            ct2.transpose(0, 2, 1).reshape(P, -1)).astype(
                ml_dtypes.float8_e4m3fn)
        wpack = np.concatenate([
            W1.reshape(KT, P, cfg.d_hid).transpose(1, 0, 2).reshape(P, -1),
            Wa1.reshape(KT, P, 2 * cfg.heads).transpose(1, 0, 2).reshape(P, -1),
            W2a.reshape(K2, P, cfg.d_out + 2).transpose(1, 0, 2).reshape(P, -1),
            b1r,
            dstidx,
            iotaT[:, :P * NGM],
        ], axis=1).astype(ml_dtypes.bfloat16)
        im = {
            "xTo": np.ascontiguousarray(
                xT[:, c * cfg.npc:(c + 1) * cfg.npc]).astype(
                    ml_dtypes.bfloat16),
            "wpack": wpack,
            "b2r": b2r.astype(np.float32),
            "gi": gi, "ct2": ct2,
        }
        if KB1 > KA:
            im["gi2"] = gi2
        in_maps.append(im)
    return in_maps, node_of_slot


def build_program(cfg: Cfg) -> bass.Bass:
    import concourse.bacc as bacc
    nc = bacc.Bacc("TRN2", target_bir_lowering=False, num_devices=cfg.n_cores)
    DH, HD, DO = cfg.d_hid, cfg.heads, cfg.d_out
    KA, KB1 = cfg.ka, cfg.kb1
    KT = cfg.d_in // P
    K2 = DH // P
    NIDX = P * cfg.sum_ng
    NGM = cfg.ng_max
    NA = cfg.na_rows
    NB1 = cfg.nb1_rows
    RW1U = cfg.rw1u
    WC_W1 = KT * DH
    WC_WA = KT * 2 * HD
    WC_W2 = K2 * (DO + 2)
    WCOLS = WC_W1 + WC_WA + WC_W2 + DH + cfg.sum_ng + P * NGM
    o_wa = WC_W1
    o_w2 = o_wa + WC_WA
    o_b1 = o_w2 + WC_W2
    o_di = o_b1 + DH
    o_io = o_di + cfg.sum_ng

    xTo = nc.dram_tensor("xTo", [cfg.d_in, cfg.npc], BF16,
                         kind="ExternalInput")
    wpk = nc.dram_tensor("wpack", [P, WCOLS], BF16, kind="ExternalInput")
    b2r = nc.dram_tensor("b2r", [P, DO], F32, kind="ExternalInput")
    gi = nc.dram_tensor("gi", [P, NIDX // 16], I16, kind="ExternalInput")
    gi2t = (nc.dram_tensor("gi2", [P, NIDX // 16], I16,
                           kind="ExternalInput") if KB1 > KA else None)
    ct2 = nc.dram_tensor("ct2", [P, NIDX], FP8, kind="ExternalInput")
    out = nc.dram_tensor("out", [cfg.npc, DO], F32, kind="ExternalOutput")

    haug = nc.dram_tensor("haug", [cfg.npad, cfg.rw1], BF16,
                          addr_space="Shared" if cfg.collective else "Local")
    h2tab = nc.dram_tensor("h2tab", [NA + (cfg.tpc - KB1) * P *
                                     cfg.n_cores, cfg.rw2], BF16,
                           addr_space="Shared" if cfg.collective else "Local")
    h2tb1 = (nc.dram_tensor("h2tb1", [NB1, cfg.rw2], BF16,
                            addr_space="Shared" if cfg.collective
                            else "Local") if KB1 > KA else None)
    if cfg.collective:
        hoa = nc.dram_tensor("hoa", [KA * P, cfg.rw1], BF16)
        hob = nc.dram_tensor("hob", [(cfg.tpc - KA) * P, cfg.rw1], BF16)
        h2oa = nc.dram_tensor("h2oa", [KA * P, cfg.rw2], BF16)
        h2ob1 = (nc.dram_tensor("h2ob1", [(KB1 - KA) * P, cfg.rw2], BF16)
                 if KB1 > KA else None)
        h2ob2 = (nc.dram_tensor("h2ob2", [(cfg.tpc - KB1) * P, cfg.rw2],
                                BF16) if cfg.tpc > KB1 else None)
    grp = [list(range(cfg.n_cores))]

    from contextlib import ExitStack
    with tile.TileContext(nc) as tc, ExitStack() as st:
        cst = st.enter_context(tc.tile_pool(name="cst", bufs=1))
        psB_p = st.enter_context(tc.tile_pool(name="psB", bufs=3,
                                              space="PSUM"))
        psS_p = st.enter_context(tc.tile_pool(name="psS", bufs=4,
                                              space="PSUM"))
        psT_p = st.enter_context(tc.tile_pool(name="psT", bufs=1,
                                              space="PSUM"))
        hga_p = st.enter_context(tc.tile_pool(name="hga", bufs=4))
        hgb1_p = st.enter_context(tc.tile_pool(name="hgb1", bufs=2))
        hgb2_p = st.enter_context(tc.tile_pool(name="hgb2", bufs=3))
        hg2_p = st.enter_context(tc.tile_pool(name="hg2", bufs=2))
        sm_p = st.enter_context(tc.tile_pool(name="sm", bufs=6))
        sm2_p = st.enter_context(tc.tile_pool(name="sm2", bufs=2))
        big_p = st.enter_context(tc.tile_pool(name="big", bufs=2))
        hsb_p = st.enter_context(tc.tile_pool(name="hsb", bufs=3))
        out_p = st.enter_context(tc.tile_pool(name="outp", bufs=3))

        xosb = cst.tile([P, KT, cfg.npc], BF16)
        wsb = cst.tile([P, WCOLS], BF16)
        b2sb = cst.tile([P, DO], F32)
        gisb = cst.tile([P, NIDX // 16], I16)
        gi2sb = cst.tile([P, NIDX // 16], I16) if KB1 > KA else gisb
        ctall = cst.tile([P, NIDX], FP8)
        c2all = cst.tile([P, P, cfg.sum_ng], BF16)
        identb = cst.tile([P, P], BF16)
        ad_bf = cst.tile([P, cfg.tpc, HD], BF16)
        ad2_bf = cst.tile([P, cfg.tpc, 1], BF16)
        ade_sb = cst.tile([P, cfg.sum_ng, HD], BF16)
        ad2e_sb = cst.tile([P, cfg.sum_ng], BF16)
        osbA = cst.tile([P, cfg.tpc, DO + 1], F32)

        h3 = cfg.npc // 2
        nc.sync.dma_start(out=xosb[:, :, :h3], in_=xTo[:, :h3].rearrange(
            "(k p) n -> p k n", p=P))
        nc.sync.dma_start(out=xosb[:, :, h3:], in_=xTo[:, h3:].rearrange(
            "(k p) n -> p k n", p=P))
        nc.sync.dma_start(out=wsb[:], in_=wpk[:])
        nc.sync.dma_start(out=gisb[:], in_=gi[:])
        if KB1 > KA:
            nc.sync.dma_start(out=gi2sb[:], in_=gi2t[:])
        nc.sync.dma_start(out=b2sb[:], in_=b2r[:])
        make_identity(nc, identb[:])

        w1sb = wsb[:, :WC_W1].rearrange("p (k d) -> p k d", k=KT)
        wa1sb = wsb[:, o_wa:o_wa + WC_WA].rearrange("p (k d) -> p k d", k=KT)
        w2sb = wsb[:, o_w2:o_w2 + WC_W2].rearrange("p (k d) -> p k d", k=K2)
        b1sb = wsb[:, o_b1:o_b1 + DH]
        disb = wsb[:, o_di:o_di + cfg.sum_ng]
        iosb = wsb[:, o_io:o_io + P * NGM].rearrange("p (d g) -> p d g", d=P)

        half = cfg.hid // 2

        def rng3(t):
            return ((cfg.offs_a(t), cfg.ga[t]),
                    (cfg.offs_b1(t), cfg.gb1[t]),
                    (cfg.offs_b2(t), cfg.gb2[t]))

        def a_tile(t):
            lt = xosb[:, :, t * P:(t + 1) * P]
            ph = psB_p.tile([P, DH], F32, tag="big")
            pa_t = psS_p.tile([P, 256], F32, tag="sm")
            pa = pa_t[:, :2 * HD]
            for k in range(KT):
                nc.tensor.matmul(ph[:], lt[:, k, :], w1sb[:, k, :],
                                 start=(k == 0), stop=(k == KT - 1))
            for k in range(KT):
                nc.tensor.matmul(pa[:], lt[:, k, :], wa1sb[:, k, :],
                                 start=(k == 0), stop=(k == KT - 1))
            hs = hsb_p.tile([P, RW1U], BF16, tag="hsb")
            hsr = hs[:, :DH].rearrange("p (c h) -> p h c", h=HD)
            phr = ph[:].rearrange("p (h c) -> p h c", h=HD)
            nc.vector.tensor_copy(hsr[:, :, :half], phr[:, :, :half])
            nc.scalar.copy(hsr[:, :, half:], phr[:, :, half:])
            nc.scalar.copy(hs[:, DH:DH + HD], pa[:, :HD])
            nc.scalar.copy(ad_bf[:, t, :], pa[:, HD:2 * HD])
            if cfg.collective:
                hdst = hoa if t < KA else hob
                r0 = (t if t < KA else t - KA) * P
            else:
                hdst = haug
                r0 = t * P if t < KA else NA + (t - KA) * P
            nc.sync.dma_start(out=hdst[r0:r0 + P, :RW1U], in_=hs[:])

        def c2_build(t):
            for off, ng in rng3(t):
                if ng == 0:
                    continue
                nc.vector.tensor_tensor(
                    out=c2all[:, :, off:off + ng],
                    in0=disb[:, off:off + ng].unsqueeze(1)
                        .broadcast_to([P, P, ng]),
                    in1=iosb[:, :, :ng],
                    op=mybir.AluOpType.is_equal)

        def ade_pre(t):
            pad_e_t = psS_p.tile([P, 256], F32, tag="sm")
            j = 0
            for off, ng in rng3(t):
                if ng == 0:
                    continue
                e0 = P * off
                nc.sync.dma_start(out=ctall[:, e0:e0 + P * ng],
                                  in_=ct2[:, e0:e0 + P * ng])
                for g in range(ng):
                    nc.tensor.matmul(
                        pad_e_t[:, (j + g) * HD:(j + g + 1) * HD],
                        ctall[:, e0 + g * P:e0 + (g + 1) * P],
                        ad_bf[:, t, :], start=True, stop=True)
                nc.scalar.copy(
                    ade_sb[:, off:off + ng, :],
                    pad_e_t[:, j * HD:(j + ng) * HD]
                    .rearrange("p (g h) -> p g h", h=HD))
                j += ng

        doB = "B" in cfg.phases
        doC = "C" in cfg.phases
        doD = "D" in cfg.phases

        def gather_a(t):
            if not cfg.ga[t]:
                return None
            hga = hga_p.tile([P, max(cfg.ga), cfg.rw1], BF16, tag="hga")
            off = cfg.offs_a(t)
            e0, nidx = P * off, P * cfg.ga[t]
            nc.gpsimd.dma_gather(
                out_ap=hga[:, :cfg.ga[t], :], in_ap=haug[0:NA, :],
                idxs_ap=gisb[:, e0 // 16:(e0 + nidx) // 16],
                num_idxs=nidx, num_idxs_reg=nidx, elem_size=cfg.rw1,
                single_packet=False)
            return hga

        # ---- phase A + AllGathers ----
        ADE_PRE = 4
        NPRE = 4
        pre_a = {}
        for t in range(KA):
            a_tile(t)
            c2_build(t)
            if t < ADE_PRE:
                ade_pre(t)
        if cfg.collective:
            nc.gpsimd.collective_compute(
                "AllGather", mybir.AluOpType.bypass,
                ins=[hoa[:]], outs=[haug[0:NA, :]], replica_groups=grp)
        if doB:
            for t in range(NPRE):
                pre_a[t] = gather_a(t)
        for t in range(KA, cfg.tpc):
            a_tile(t)
            c2_build(t)
        if cfg.collective:
            nc.gpsimd.collective_compute(
                "AllGather", mybir.AluOpType.bypass,
                ins=[hob[:]], outs=[haug[NA:, :]], replica_groups=grp)

        # ---- phase B ----
        def b_front(t):
            parts = []
            if cfg.ga[t]:
                hga = pre_a.pop(t) if t in pre_a else gather_a(t)
                parts.append((cfg.offs_a(t), cfg.ga[t], hga))
            for q, gl, pool, tg in ((1, cfg.gb1, hgb1_p, "hgb1"),
                                    (2, cfg.gb2, hgb2_p, "hgb2")):
                if not gl[t]:
                    continue
                buf = pool.tile([P, max(gl), cfg.rw1], BF16, tag=tg)
                off = cfg.offs_b1(t) if q == 1 else cfg.offs_b2(t)
                e0, nidx = P * off, P * gl[t]
                nc.gpsimd.dma_gather(
                    out_ap=buf[:, :gl[t], :], in_ap=haug[:, :],
                    idxs_ap=gisb[:, e0 // 16:(e0 + nidx) // 16],
                    num_idxs=nidx, num_idxs_reg=nidx, elem_size=cfg.rw1,
                    single_packet=False)
                parts.append((off, gl[t], buf))
            psD_t = psS_p.tile([P, 256], F32, tag="sm")
            psY = psB_p.tile([P, DH], F32, tag="big")
            ngtot = sum(ng for _, ng, _ in parts)
            # batched per engine: DVE adds, ACT prelu+exp, PE den-mm,
            # DVE mults, PE psY-mm
            for off, ng, buf in parts:
                ex = buf[:, :ng, DH:DH + HD]
                nc.vector.tensor_tensor(
                    out=ex, in0=ex, in1=ade_sb[:, off:off + ng, :],
                    op=mybir.AluOpType.add)
            for off, ng, buf in parts:
                ex = buf[:, :ng, DH:DH + HD]
                nc.scalar.activation(ex, ex,
                                     mybir.ActivationFunctionType.Prelu,
                                     alpha=NEG_SLOPE)
                nc.scalar.activation(ex, ex,
                                     mybir.ActivationFunctionType.Exp)
            gdone = 0
            for off, ng, buf in parts:
                hgv = buf[:, :ng, :]
                for g in range(ng):
                    nc.tensor.matmul(psD_t[:, :HD],
                                     c2all[:, :, off + g],
                                     hgv[:, g, DH:DH + HD],
                                     start=(gdone + g == 0),
                                     stop=(gdone + g == ngtot - 1),
                                     skip_group_check=True)
                gdone += ng
            for off, ng, buf in parts:
                hgv = buf[:, :ng, :]
                hgm = hgv[:, :, :DH].rearrange("p g (c h) -> p g c h", h=HD)
                ex = hgv[:, :, DH:DH + HD]
                nc.vector.tensor_tensor(
                    out=hgm, in0=hgm,
                    in1=ex.unsqueeze(2).broadcast_to([P, ng, cfg.hid, HD]),
                    op=mybir.AluOpType.mult)
            gdone = 0
            for off, ng, buf in parts:
                hgv = buf[:, :ng, :]
                for g in range(ng):
                    nc.tensor.matmul(psY[:], c2all[:, :, off + g],
                                     hgv[:, g, :DH],
                                     start=(gdone + g == 0),
                                     stop=(gdone + g == ngtot - 1),
                                     skip_group_check=True)
                gdone += ng
            return psY, psD_t

        def b_back(t, psY, psD):
            den = sm_p.tile([P, HD], F32, tag="den")
            nc.vector.tensor_scalar_max(out=den[:], in0=psD[:, :HD],
                                        scalar1=1e-30)
            rec = sm_p.tile([P, HD], F32, tag="rec")
            nc.vector.reciprocal(rec[:], den[:])
            y = big_p.tile([P, DH], BF16, tag="y")
            nc.vector.tensor_tensor(
                out=y[:].rearrange("p (c h) -> p c h", h=HD),
                in0=psY[:].rearrange("p (c h) -> p c h", h=HD),
                in1=rec[:].unsqueeze(1).broadcast_to([P, cfg.hid, HD]),
                op=mybir.AluOpType.mult)
            nc.vector.tensor_add(y[:], y[:], b1sb[:])
            tneg = big_p.tile([P, DH], BF16, tag="tneg")
            nc.scalar.activation(tneg[:], y[:],
                                 mybir.ActivationFunctionType.Relu, scale=-1.0)
            nc.scalar.activation(y[:], y[:],
                                 mybir.ActivationFunctionType.Relu)
            nc.scalar.activation(tneg[:], tneg[:],
                                 mybir.ActivationFunctionType.Exp, scale=-1.0)
            nc.vector.scalar_tensor_tensor(
                out=y[:], in0=tneg[:], scalar=-1.0, in1=y[:],
                op0=mybir.AluOpType.add, op1=mybir.AluOpType.add)
            if not doC:
                return
            pt = psT_p.tile([P, K2, P], BF16, tag="pt")
            for k in range(K2):
                nc.tensor.transpose(pt[:, k, :], y[:, k * P:(k + 1) * P],
                                    identb[:])
            yT = big_p.tile([P, K2, P], BF16, tag="yT")
            nc.scalar.copy(yT[:], pt[:])
            p2_t = psS_p.tile([P, 256], F32, tag="sm")
            p2 = p2_t[:, :DO + 2]
            for k in range(K2):
                nc.tensor.matmul(p2[:], yT[:, k, :], w2sb[:, k, :],
                                 start=(k == 0), stop=(k == K2 - 1))
            h2sb = out_p.tile([P, DO + 1], BF16, tag="h2sb")
            nc.scalar.copy(h2sb[:], p2[:, :DO + 1])
            nc.scalar.copy(ad2_bf[:, t, :], p2[:, DO + 1:DO + 2])
            if cfg.collective:
                if t < KA:
                    h2dst, r0 = h2oa, t * P
                elif t < KB1:
                    h2dst, r0 = h2ob1, (t - KA) * P
                else:
                    h2dst, r0 = h2ob2, (t - KB1) * P
            else:
                if t < KA:
                    h2dst, r0 = h2tab, t * P
                elif t < KB1:
                    h2dst, r0 = h2tb1, (t - KA) * P
                else:
                    h2dst, r0 = h2tab, NA + (t - KB1) * P
            nc.sync.dma_start(out=h2dst[r0:r0 + P, :DO + 1], in_=h2sb[:])

        def pad2_pre(t):
            pq_t = psS_p.tile([P, 256], F32, tag="sm")
            j = 0
            for off, ng in rng3(t):
                for g in range(ng):
                    nc.tensor.matmul(
                        pq_t[:, j + g:j + g + 1],
                        ctall[:, P * (off + g):P * (off + g + 1)],
                        ad2_bf[:, t, :], start=True, stop=True)
                j += ng
            j = 0
            for off, ng in rng3(t):
                if ng:
                    nc.scalar.copy(ad2e_sb[:, off:off + ng],
                                   pq_t[:, j:j + ng])
                j += ng

        # ---- phase D helpers ----
        def d_gather(ts, q, buf):
            gl = (cfg.ga, cfg.gb1, cfg.gb2)[q]
            ngsum = sum(gl[t] for t in ts)
            if ngsum == 0:
                return
            off0 = (cfg.offs_a(ts[0]), cfg.offs_b1(ts[0]),
                    cfg.offs_b2(ts[0]))[q]
            e0 = P * off0
            nidx = P * ngsum
            src = (h2tab[0:NA, :],
                   h2tb1[:, :] if h2tb1 is not None else None,
                   h2tab[:, :])[q]
            nc.gpsimd.dma_gather(
                out_ap=buf[:, :ngsum, :], in_ap=src,
                idxs_ap=gi2sb[:, e0 // 16:(e0 + nidx) // 16],
                num_idxs=nidx, num_idxs_reg=nidx, elem_size=cfg.rw2,
                single_packet=False)

        def d_chunk(ts, q, buf, psO):
            """ex2 chain + scale + per-tile scatter for one gathered chunk.
            Per-tile psum regions: q==0 standalone; q==1 opens, q==2 closes
            (flags adapt when a tile has no groups in one of the ranges)."""
            gl = (cfg.ga, cfg.gb1, cfg.gb2)[q]
            ngs = [gl[t] for t in ts]
            ngsum = sum(ngs)
            if ngsum == 0:
                return
            off0 = (cfg.offs_a(ts[0]), cfg.offs_b1(ts[0]),
                    cfg.offs_b2(ts[0]))[q]
            hv = buf[:, :ngsum, :]
            ex2 = hv[:, :, DO:DO + 1]
            nc.vector.tensor_tensor(
                out=ex2, in0=ex2,
                in1=ad2e_sb[:, off0:off0 + ngsum].unsqueeze(2),
                op=mybir.AluOpType.add)
            nc.scalar.activation(ex2, ex2,
                                 mybir.ActivationFunctionType.Prelu,
                                 alpha=NEG_SLOPE)
            nc.scalar.activation(ex2, ex2, mybir.ActivationFunctionType.Exp)
            nc.vector.tensor_tensor(
                out=hv[:, :, :DO], in0=hv[:, :, :DO],
                in1=ex2.broadcast_to([P, ngsum, DO]),
                op=mybir.AluOpType.mult)
            co = 0
            for i, t in enumerate(ts):
                off = (cfg.offs_a(t), cfg.offs_b1(t), cfg.offs_b2(t))[q]
                if q == 0:
                    st_t, sp_t = True, True
                else:
                    st_t, sp_t = True, True
                for g in range(ngs[i]):
                    nc.tensor.matmul(
                        psO[:, i * (DO + 1):(i + 1) * (DO + 1)],
                        c2all[:, :, off + g], hv[:, co + g, :DO + 1],
                        start=(st_t and g == 0),
                        stop=(sp_t and g == ngs[i] - 1),
                        skip_group_check=True)
                co += ngs[i]

        def d_finish(ts, psOb):
            n = len(ts)
            t0 = ts[0]
            osb_ch = out_p.tile([P, cfg.tpc // 2, DO], F32, tag="osbch")
            # dense ACT copy of the chunk psums to SBUF, then chunk-wide
            # math (strided SBUF reads are exact; strided PSUM is not)
            stg = sm2_p.tile([P, (cfg.tpc // 2) * (DO + 1)], F32,
                             tag="fstg")
            nc.scalar.copy(stg[:, :n * (DO + 1)], psOb[:, :n * (DO + 1)])
            pr = stg[:, :n * (DO + 1)].rearrange("p (i d) -> p i d",
                                                 d=DO + 1)
            tot = sm2_p.tile([P, cfg.tpc // 2, DO + 1], F32, tag="tot")
            nc.vector.tensor_tensor(out=tot[:, :n, :], in0=pr[:],
                                    in1=osbA[:, t0:t0 + n, :],
                                    op=mybir.AluOpType.add)
            if KB1 > KA:
                nc.vector.tensor_tensor(out=tot[:, :n, :],
                                        in0=tot[:, :n, :],
                                        in1=osbB[:, t0:t0 + n, :],
                                        op=mybir.AluOpType.add)
            den2 = sm2_p.tile([P, cfg.tpc // 2], F32, tag="den2")
            rec2 = sm2_p.tile([P, cfg.tpc // 2], F32, tag="rec2")
            nc.vector.tensor_scalar_max(out=den2[:, :n],
                                        in0=tot[:, :n, DO],
                                        scalar1=1e-30)
            nc.vector.reciprocal(rec2[:, :n], den2[:, :n])
            for i, t in enumerate(ts):
                nc.vector.scalar_tensor_tensor(
                    out=osb_ch[:, i, :], in0=tot[:, i, :DO],
                    scalar=rec2[:, i:i + 1], in1=b2sb[:],
                    op0=mybir.AluOpType.mult, op1=mybir.AluOpType.add)
            nc.sync.dma_start(
                out=out[t0 * P:(t0 + n) * P, :].rearrange(
                    "(i p) d -> p i d", p=P),
                in_=osb_ch[:, :n, :])

        CH = [list(range(0, 5)), list(range(5, cfg.tpc))]
        CH2MAX = max(max(sum(cfg.ga[t] for t in ts) for ts in CH),
                     max(sum(cfg.gb1[t] for t in ts) for ts in CH),
                     max(sum(cfg.gb2[t] for t in ts) for ts in CH), 1)

        def ag_h2(block):
            if not cfg.collective:
                return
            if block == "b1" and KB1 == KA:
                return
            if block == "b2" and cfg.tpc == KB1:
                return
            ins_, outt, lo, hi = {
                "a": (h2oa, h2tab, 0, NA),
                "b1": (h2ob1, h2tb1, 0, NB1),
                "b2": (h2ob2, h2tab, NA,
                       NA + (cfg.tpc - KB1) * P * cfg.n_cores),
            }[block]
            nc.gpsimd.collective_compute(
                "AllGather", mybir.AluOpType.bypass,
                ins=[ins_[:]], outs=[outt[lo:hi, :]], replica_groups=grp)

        if doB:
            state = {}
            dbufs = {}
            for t in range(cfg.tpc):
                if ADE_PRE + t < cfg.tpc:
                    ade_pre(ADE_PRE + t)
                state[t] = b_front(t)
                if t >= 1:
                    b_back(t - 1, *state.pop(t - 1))
                    if doD:
                        pad2_pre(t - 1)
            if doD and doC:
                # AG-a + D-a gathers queue behind the last B gathers
                ag_h2("a")
                for ci, ts in enumerate(CH):
                    buf = hg2_p.tile([P, CH2MAX, cfg.rw2], BF16, tag="hg2")
                    dbufs[("a", ci)] = buf
                    d_gather(ts, 0, buf)
            b_back(cfg.tpc - 1, *state.pop(cfg.tpc - 1))
            if doD:
                pad2_pre(cfg.tpc - 1)

        if doB and doC and doD:
            psOs = {}
            # a-chunk computes (data arrived during B tail) -> osbA;
            # then b1 gathers reuse the a-buffers (read-before-write order)
            for ci, ts in enumerate(CH):
                psOa = psS_p.tile([P, 256], F32, tag="sm")
                if sum(cfg.ga[t] for t in ts):
                    d_chunk(ts, 0, dbufs[("a", ci)], psOa)
                n = len(ts)
                nc.scalar.copy(
                    osbA[:, ts[0]:ts[0] + n, :],
                    psOa[:, :n * (DO + 1)].rearrange(
                        "p (i d) -> p i d", d=DO + 1))
                for t in ts:
                    if cfg.ga[t] == 0:
                        nc.vector.memset(osbA[:, t, :], 0.0)
                if ci == 0:
                    ag_h2("b1")
                buf = hg2_p.tile([P, CH2MAX, cfg.rw2], BF16, tag="hg2")
                dbufs[("b1", ci)] = buf
                d_gather(ts, 1, buf)


            ag_h2("b2")
            osbB = cst.tile([P, cfg.tpc, DO + 1], F32) if KB1 > KA else None
            for ci, ts in enumerate(CH):
                n = len(ts)
                if sum(cfg.gb1[t] for t in ts):
                    psOb = psS_p.tile([P, 256], F32, tag="sm")
                    d_chunk(ts, 1, dbufs[("b1", ci)], psOb)
                    nc.scalar.copy(
                        osbB[:, ts[0]:ts[0] + n, :],
                        psOb[:, :n * (DO + 1)].rearrange(
                            "p (i d) -> p i d", d=DO + 1))
                    for t in ts:
                        if cfg.gb1[t] == 0:
                            nc.vector.memset(osbB[:, t, :], 0.0)
                elif KB1 > KA:
                    nc.vector.memset(osbB[:, ts[0]:ts[0] + n, :], 0.0)
                buf = hg2_p.tile([P, CH2MAX, cfg.rw2], BF16, tag="hg2")
                dbufs[("b2", ci)] = buf
                d_gather(ts, 2, buf)
            import os as _osd
            if _osd.environ.get("DBG_AB"):
                dbgA = nc.dram_tensor("dbgA", [P, cfg.tpc * (DO + 1)], F32,
                                      kind="ExternalOutput")
                nc.sync.dma_start(out=dbgA[:],
                                  in_=osbA[:].rearrange("p a b -> p (a b)"))
                dbgB = nc.dram_tensor("dbgB", [P, cfg.tpc * (DO + 1)], F32,
                                      kind="ExternalOutput")
                nc.sync.dma_start(out=dbgB[:],
                                  in_=osbB[:].rearrange("p a b -> p (a b)"))
            for ci, ts in enumerate(CH):
                psOb2 = psS_p.tile([P, 256], F32, tag="sm")
                psOs[ci] = psOb2
                d_chunk(ts, 2, dbufs[("b2", ci)], psOb2)
                d_finish(ts, psOs[ci])

    nc.compile()
    return nc


def default_cfg() -> Cfg:
    return Cfg()


def run(inputs: dict, cfg: Cfg | None = None, **run_kwargs):
    cfg = cfg or default_cfg()
    in_maps, node_of_slot = preprocess(cfg, **inputs)
    nc = build_program(cfg)
    res = run_bass_kernel_spmd(nc, in_maps, list(range(cfg.n_cores)),
                               **run_kwargs)
    outs = np.concatenate([res.results[c]["out"] for c in range(cfg.n_cores)],
                          axis=0)
    full = np.zeros((cfg.n_nodes, cfg.d_out), np.float32)
    flat = node_of_slot.reshape(-1)
    real = flat >= 0
    full[flat[real]] = outs[real]
    return full, res


def kernel(**inputs) -> np.ndarray:
    out, _ = run(inputs)
    return out


# revision 80
# speedup vs baseline: 1.0013x; 1.0013x over previous
"""Bass/Trainium2 kernel v4 for 2-layer GAT (nn_GAT_48919677501958).

Contract: kernel(**inputs) takes FULL unsharded numpy inputs, returns the
FULL [10000, 40] float32 output.

v4 strategy: 3-block split-table pipelining + fp8 one-hots.
  - Nodes are dealt to 8 cores x 10 tiles of 128 dsts.  The shared layer-1
    table `haug` has 2 blocks (a = every core's tiles [0,6), b = tiles
    [6,10)); the layer-2 table `h2tab` has 3 blocks (a = tiles [0,6),
    b1 = [6,8), b2 = [8,10)).  Each block is AllGather'ed separately into
    slices of one tensor, so
      * B-phase gathers of early-src edges start right after A finishes its
        first 6 tiles,
      * D-phase gathers of a-srcs run while the B pipeline drains, b1-srcs
        while the last two b_backs run, and only the b2 srcs (~20% of
        edges) wait for the final tile.
  - Per (core, tile) the edge list is split into aligned a/b1/b2 ranges
    (SPMD-uniform group counts; spill goes to later ranges, padding
    minimized by search).  B gathers 3 parts per tile; D gathers 2 big
    chunks per block.
  - ct2 (the [dst, edge] one-hot for PE broadcasts) is fp8 (exact 0/1).
  - exp(leaky_relu) is computed as Prelu(alpha) -> Exp on ACT (same HW
    activation-function set, no table reload).
  - haug rows are written as 520 used elems (no pad write); constants are
    packed into one DRAM blob; chain ops are issued batched per engine.
"""

from dataclasses import dataclass, field

import numpy as np

import concourse.bass as bass
import concourse.mybir as mybir
import concourse.tile as tile
from concourse.bass_utils import run_bass_kernel_spmd
from concourse.masks import make_identity

F32 = mybir.dt.float32
BF16 = mybir.dt.bfloat16
FP8 = mybir.dt.float8e4
I16 = mybir.dt.int16

NEG_SLOPE = 0.2
P = 128


@dataclass
class Cfg:
    n_nodes: int = 10000
    n_cores: int = 8
    tpc: int = 10
    ka: int = 6   # tiles [0,ka) form block-a
    kb1: int = 6  # tiles [ka,kb1) form block-b1; rest b2
    d_in: int = 256
    hid: int = 64
    heads: int = 8
    d_out: int = 40
    ga: list[int] = field(default_factory=list)
    gb1: list[int] = field(default_factory=list)
    gb2: list[int] = field(default_factory=list)
    collective: bool = True
    phases: str = "ABCD"

    @property
    def npc(self):
        return self.tpc * P

    @property
    def npad(self):
        return self.n_cores * self.npc

    @property
    def na_rows(self):  # rows in block-a
        return self.n_cores * self.ka * P

    @property
    def nb1_rows(self):
        return self.n_cores * (self.kb1 - self.ka) * P

    @property
    def d_hid(self):
        return self.hid * self.heads

    @property
    def rw1(self):  # table-1 row elems: 512 h + 8 alpha_src + pad (256B rule)
        return 640

    @property
    def rw1u(self):
        return self.d_hid + self.heads

    @property
    def rw2(self):
        return 128

    @property
    def sum_ng(self):
        return sum(self.ga) + sum(self.gb1) + sum(self.gb2)

    @property
    def ng_max(self):
        return max(max(self.ga, default=1), max(self.gb1, default=1),
                   max(self.gb2, default=1), 1)

    # global column order: [t0a..t9a | t0b1..t9b1 | t0b2..t9b2]
    def offs_a(self, t):
        return sum(self.ga[:t])

    def offs_b1(self, t):
        return sum(self.ga) + sum(self.gb1[:t])

    def offs_b2(self, t):
        return sum(self.ga) + sum(self.gb1) + sum(self.gb2[:t])


def _wrap_idx(flat: np.ndarray) -> np.ndarray:
    assert flat.size % 16 == 0
    w = np.ascontiguousarray(flat.reshape(-1, 16).T).astype(np.int16)
    return np.tile(w, (8, 1))


def _split_groups(cnt_first, cnt_rest):
    """groups for the first range (tie-max pad-min), spilling into rest."""
    P_ = P
    best = None
    for g in range(0, int(cnt_first.max()) // P_ + 1):
        placed = np.minimum(cnt_first, g * P_)
        rest = cnt_first - placed + cnt_rest
        grest = int(np.ceil(rest.max() / P_)) if rest.max() else 0
        pad = (g * P_ - placed).sum() + (grest * P_ - rest).sum()
        if best is None or pad <= best[0]:
            best = (pad, g, grest)
    return best[1]


def preprocess(cfg: Cfg, x, edge_index, W1, att_src1, att_dst1, b1, W2,
               att_src2, att_dst2, b2):
    import ml_dtypes
    N = cfg.n_nodes
    KA, KB1 = cfg.ka, cfg.kb1
    NA = cfg.na_rows
    src = np.concatenate([np.asarray(edge_index[0], np.int64), np.arange(N)])
    dst = np.concatenate([np.asarray(edge_index[1], np.int64), np.arange(N)])
    deg = np.bincount(dst, minlength=N)

    order_e = np.argsort(dst, kind="stable")
    sorted_src = src[order_e]
    starts = np.zeros(N + 1, np.int64)
    np.cumsum(deg, out=starts[1:])

    node_order = np.argsort(-deg, kind="stable")
    n_tiles = cfg.n_cores * cfg.tpc
    tiles = np.full((n_tiles, P), -1, np.int64)
    wts = np.array([0.42] + [1.13] * (cfg.tpc - 3) + [0.82, 0.42])

    for c in range(cfg.n_cores):
        mine = node_order[c::cfg.n_cores]
        tgt = wts / wts.sum() * deg[mine].sum()
        cur = np.zeros(cfg.tpc)
        cnt = np.zeros(cfg.tpc, np.int64)
        cap = 18 * P - 16
        for n in mine:
            free = cnt < P
            ok = free & (cur + deg[n] <= cap)
            pick = ok if ok.any() else free
            t = int(np.argmax(np.where(pick, tgt - cur, -np.inf)))
            tiles[c * cfg.tpc + t, cnt[t]] = n
            cur[t] += deg[n]
            cnt[t] += 1

    node_of_slot = np.full((cfg.n_cores, cfg.npc), -1, np.int64)
    for c in range(cfg.n_cores):
        for t in range(cfg.tpc):
            node_of_slot[c, t * P:(t + 1) * P] = tiles[c * cfg.tpc + t]
    flat_slots = node_of_slot.reshape(-1)
    real = flat_slots >= 0
    slot_of_node = np.full(N, -1, np.int64)
    slot_of_node[flat_slots[real]] = np.nonzero(real)[0]
    assert (slot_of_node >= 0).all()

    # table rows per slot for both layers
    slot_arr = np.arange(cfg.npad)
    c_arr, r_arr = slot_arr // cfg.npc, slot_arr % cfg.npc
    t_arr, p_arr = r_arr // P, r_arr % P
    # layer-1 (haug): a-block + one b-block (c-major over tiles [KA,10))
    trow1 = np.where(
        t_arr < KA, c_arr * (KA * P) + t_arr * P + p_arr,
        NA + c_arr * ((cfg.tpc - KA) * P) + (t_arr - KA) * P + p_arr)
    # layer-2 (h2tab): a, b1, b2 blocks
    nb1t = KB1 - KA
    nb2t = cfg.tpc - KB1
    trow2 = np.where(
        t_arr < KA, c_arr * (KA * P) + t_arr * P + p_arr,
        np.where(
            t_arr < KB1,
            c_arr * (nb1t * P) + (t_arr - KA) * P + p_arr,  # local to h2tb1
            NA + c_arr * (nb2t * P) + (t_arr - KB1) * P + p_arr))
    rng_of_slot = np.where(t_arr < KA, 0, np.where(t_arr < KB1, 1, 2))

    # per (core, tile): edges split into 3 src-ranges, dst-major
    ed = [[None] * cfg.tpc for _ in range(cfg.n_cores)]
    for c in range(cfg.n_cores):
        for t in range(cfg.tpc):
            parts = [[[], [], []] for _ in range(3)]  # rng -> [r1, r2, dloc]
            for d in range(P):
                n = node_of_slot[c, t * P + d]
                if n < 0:
                    continue
                ss = sorted_src[starts[n]:starts[n] + deg[n]]
                sl = slot_of_node[ss]
                rg = rng_of_slot[sl]
                for q in range(3):
                    m = rg == q
                    parts[q][0].append(trow1[sl[m]])
                    parts[q][1].append(trow2[sl[m]])
                    parts[q][2].append(np.full(int(m.sum()), d, np.int64))
            ed[c][t] = [tuple(np.concatenate(pp) if pp else
                              np.zeros(0, np.int64) for pp in parts[q])
                        for q in range(3)]

    # choose group counts: a (spill->b1), b1 (spill->b2), b2 = ceil rest
    cfg.ga, cfg.gb1, cfg.gb2 = [], [], []
    for t in range(cfg.tpc):
        n0 = np.array([ed[c][t][0][0].size for c in range(cfg.n_cores)])
        n1 = np.array([ed[c][t][1][0].size for c in range(cfg.n_cores)])
        n2 = np.array([ed[c][t][2][0].size for c in range(cfg.n_cores)])
        ga = _split_groups(n0, n1 + n2)
        sp0 = n0 - np.minimum(n0, ga * P)
        if KB1 > KA:
            gb1 = int(np.ceil((sp0 + n1).max() / P)) if (sp0 + n1).max() \
                else 0
            gb2 = int(np.ceil(n2.max() / P)) if n2.max() else 0
        else:
            gb1 = 0
            gb2 = int(np.ceil((sp0 + n2).max() / P)) if (sp0 + n2).max() \
                else 0
        cfg.ga.append(ga)
        cfg.gb1.append(gb1)
        cfg.gb2.append(gb2)

    # weights / constants
    ablk_s = np.zeros((cfg.d_hid, cfg.heads), np.float32)
    ablk_d = np.zeros((cfg.d_hid, cfg.heads), np.float32)
    a_s1 = np.asarray(att_src1, np.float32)
    a_d1 = np.asarray(att_dst1, np.float32)
    for h in range(cfg.heads):
        ablk_s[h * cfg.hid:(h + 1) * cfg.hid, h] = a_s1[h]
        ablk_d[h * cfg.hid:(h + 1) * cfg.hid, h] = a_d1[h]
    W1 = np.asarray(W1, np.float32)
    Wa1 = np.concatenate([W1 @ ablk_s, W1 @ ablk_d], axis=1)
    W2 = np.asarray(W2, np.float32)
    w2s = W2 @ np.asarray(att_src2, np.float32)[0]
    w2d = W2 @ np.asarray(att_dst2, np.float32)[0]
    W2a = np.concatenate([W2, w2s[:, None], w2d[:, None]], axis=1)
    b1r = np.tile(np.asarray(b1, np.float32)[None, :], (P, 1))
    b2r = np.tile(np.asarray(b2, np.float32)[None, :], (P, 1))
    j = np.arange(cfg.d_hid)
    old = (j % cfg.heads) * cfg.hid + j // cfg.heads
    b1r = np.ascontiguousarray(b1r[:, old])
    W2a = np.ascontiguousarray(W2a[old, :])

    NGM = cfg.ng_max
    iotaT = np.repeat(np.arange(P, dtype=np.float32), NGM)
    iotaT = np.tile(iotaT[None, :], (P, 1))

    xT = np.zeros((cfg.d_in, cfg.npad), np.float32)
    xT[:, np.nonzero(real)[0]] = np.asarray(x, np.float32).T[:, flat_slots[real]]

    KT = cfg.d_in // P
    K2 = cfg.d_hid // P
    in_maps = []
    for c in range(cfg.n_cores):
        g1_parts, g2_parts, di_parts = [], [], []
        for q, gl in ((0, cfg.ga), (1, cfg.gb1), (2, cfg.gb2)):
            for t in range(cfg.tpc):
                ns = gl[t] * P
                r1 = np.zeros(ns, np.int64)
                r2 = np.zeros(ns, np.int64)
                dl = np.full(ns, 255, np.int64)
                e0, e1, e2 = ed[c][t]
                na = cfg.ga[t] * P
                nb = cfg.gb1[t] * P
                sp0 = max(0, e0[0].size - na)  # a-spill count
                if q == 0:
                    pools = [(e0, 0, min(e0[0].size, na))]
                elif q == 1:
                    pools = ([(e0, e0[0].size - sp0, e0[0].size),
                              (e1, 0, e1[0].size)] if KB1 > KA else [])
                elif KB1 > KA:
                    pools = [(e2, 0, e2[0].size)]
                else:
                    pools = [(e0, e0[0].size - sp0, e0[0].size),
                             (e2, 0, e2[0].size)]
                z = [np.zeros(0, np.int64)]
                rr1 = np.concatenate([p[0][lo:hi] for p, lo, hi in pools]
                                     or z)
                rr2 = np.concatenate([p[1][lo:hi] for p, lo, hi in pools]
                                     or z)
                rrd = np.concatenate([p[2][lo:hi] for p, lo, hi in pools]
                                     or z)
                assert rr1.size <= ns, (q, t, rr1.size, ns)
                r1[:rr1.size] = rr1
                r2[:rr2.size] = rr2
                dl[:rrd.size] = rrd
                g1_parts.append(r1)
                g2_parts.append(r2)
                di_parts.append(dl.reshape(gl[t], P).T if gl[t] else
                                np.zeros((P, 0), np.int64))
        gi = _wrap_idx(np.concatenate(g1_parts))
        gi2 = _wrap_idx(np.concatenate(g2_parts))
        dstidx = np.concatenate(di_parts, axis=1).astype(np.float32)
        ct2 = (dstidx[None, :, :] == np.arange(P)[:, None, None])
        ct2 = np.ascontiguousarray(
            ct2.transpose(0, 2, 1).reshape(P, -1)).astype(
                ml_dtypes.float8_e4m3fn)
        wpack = np.concatenate([
            W1.reshape(KT, P, cfg.d_hid).transpose(1, 0, 2).reshape(P, -1),
            Wa1.reshape(KT, P, 2 * cfg.heads).transpose(1, 0, 2).reshape(P, -1),
            W2a.reshape(K2, P, cfg.d_out + 2).transpose(1, 0, 2).reshape(P, -1),
            b1r,
            dstidx,
            iotaT[:, :P * NGM],
        ], axis=1).astype(ml_dtypes.bfloat16)
        im = {
            "xTo": np.ascontiguousarray(
                xT[:, c * cfg.npc:(c + 1) * cfg.npc]).astype(
                    ml_dtypes.bfloat16),
            "wpack": wpack,
            "b2r": b2r.astype(np.float32),
            "gi": gi, "ct2": ct2,
        }
        if KB1 > KA:
            im["gi2"] = gi2
        in_maps.append(im)
    return in_maps, node_of_slot


def build_program(cfg: Cfg) -> bass.Bass:
    import concourse.bacc as bacc
    nc = bacc.Bacc("TRN2", target_bir_lowering=False, num_devices=cfg.n_cores)
    DH, HD, DO = cfg.d_hid, cfg.heads, cfg.d_out
    KA, KB1 = cfg.ka, cfg.kb1
    KT = cfg.d_in // P
    K2 = DH // P
    NIDX = P * cfg.sum_ng
    NGM = cfg.ng_max
    NA = cfg.na_rows
    NB1 = cfg.nb1_rows
    RW1U = cfg.rw1u
    WC_W1 = KT * DH
    WC_WA = KT * 2 * HD
    WC_W2 = K2 * (DO + 2)
    WCOLS = WC_W1 + WC_WA + WC_W2 + DH + cfg.sum_ng + P * NGM
    o_wa = WC_W1
    o_w2 = o_wa + WC_WA
    o_b1 = o_w2 + WC_W2
    o_di = o_b1 + DH
    o_io = o_di + cfg.sum_ng

    xTo = nc.dram_tensor("xTo", [cfg.d_in, cfg.npc], BF16,
                         kind="ExternalInput")
    wpk = nc.dram_tensor("wpack", [P, WCOLS], BF16, kind="ExternalInput")
    b2r = nc.dram_tensor("b2r", [P, DO], F32, kind="ExternalInput")
    gi = nc.dram_tensor("gi", [P, NIDX // 16], I16, kind="ExternalInput")
    gi2t = (nc.dram_tensor("gi2", [P, NIDX // 16], I16,
                           kind="ExternalInput") if KB1 > KA else None)
    ct2 = nc.dram_tensor("ct2", [P, NIDX], FP8, kind="ExternalInput")
    out = nc.dram_tensor("out", [cfg.npc, DO], F32, kind="ExternalOutput")

    haug = nc.dram_tensor("haug", [cfg.npad, cfg.rw1], BF16,
                          addr_space="Shared" if cfg.collective else "Local")
    h2tab = nc.dram_tensor("h2tab", [NA + (cfg.tpc - KB1) * P *
                                     cfg.n_cores, cfg.rw2], BF16,
                           addr_space="Shared" if cfg.collective else "Local")
    h2tb1 = (nc.dram_tensor("h2tb1", [NB1, cfg.rw2], BF16,
                            addr_space="Shared" if cfg.collective
                            else "Local") if KB1 > KA else None)
    if cfg.collective:
        hoa = nc.dram_tensor("hoa", [KA * P, cfg.rw1], BF16)
        hob = nc.dram_tensor("hob", [(cfg.tpc - KA) * P, cfg.rw1], BF16)
        h2oa = nc.dram_tensor("h2oa", [KA * P, cfg.rw2], BF16)
        h2ob1 = (nc.dram_tensor("h2ob1", [(KB1 - KA) * P, cfg.rw2], BF16)
                 if KB1 > KA else None)
        h2ob2 = (nc.dram_tensor("h2ob2", [(cfg.tpc - KB1) * P, cfg.rw2],
                                BF16) if cfg.tpc > KB1 else None)
    grp = [list(range(cfg.n_cores))]

    from contextlib import ExitStack
    with tile.TileContext(nc) as tc, ExitStack() as st:
        cst = st.enter_context(tc.tile_pool(name="cst", bufs=1))
        psB_p = st.enter_context(tc.tile_pool(name="psB", bufs=3,
                                              space="PSUM"))
        psS_p = st.enter_context(tc.tile_pool(name="psS", bufs=3,
                                              space="PSUM"))
        psT_p = st.enter_context(tc.tile_pool(name="psT", bufs=2,
                                              space="PSUM"))
        hga_p = st.enter_context(tc.tile_pool(name="hga", bufs=4))
        hgb1_p = st.enter_context(tc.tile_pool(name="hgb1", bufs=2))
        hgb2_p = st.enter_context(tc.tile_pool(name="hgb2", bufs=3))
        hg2_p = st.enter_context(tc.tile_pool(name="hg2", bufs=2))
        sm_p = st.enter_context(tc.tile_pool(name="sm", bufs=6))
        sm2_p = st.enter_context(tc.tile_pool(name="sm2", bufs=2))
        big_p = st.enter_context(tc.tile_pool(name="big", bufs=2))
        hsb_p = st.enter_context(tc.tile_pool(name="hsb", bufs=3))
        out_p = st.enter_context(tc.tile_pool(name="outp", bufs=3))

        xosb = cst.tile([P, KT, cfg.npc], BF16)
        wsb = cst.tile([P, WCOLS], BF16)
        b2sb = cst.tile([P, DO], F32)
        gisb = cst.tile([P, NIDX // 16], I16)
        gi2sb = cst.tile([P, NIDX // 16], I16) if KB1 > KA else gisb
        ctall = cst.tile([P, NIDX], FP8)
        c2all = cst.tile([P, P, cfg.sum_ng], BF16)
        identb = cst.tile([P, P], BF16)
        ad_bf = cst.tile([P, cfg.tpc, HD], BF16)
        ad2_bf = cst.tile([P, cfg.tpc, 1], BF16)
        ade_sb = cst.tile([P, cfg.sum_ng, HD], BF16)
        ad2e_sb = cst.tile([P, cfg.sum_ng], BF16)
        osbA = cst.tile([P, cfg.tpc, DO + 1], F32)

        h3 = cfg.npc // 2
        nc.sync.dma_start(out=xosb[:, :, :h3], in_=xTo[:, :h3].rearrange(
            "(k p) n -> p k n", p=P))
        nc.sync.dma_start(out=wsb[:], in_=wpk[:])
        nc.sync.dma_start(out=xosb[:, :, h3:], in_=xTo[:, h3:].rearrange(
            "(k p) n -> p k n", p=P))
        nc.sync.dma_start(out=gisb[:], in_=gi[:])
        if KB1 > KA:
            nc.sync.dma_start(out=gi2sb[:], in_=gi2t[:])
        nc.sync.dma_start(out=b2sb[:], in_=b2r[:])
        make_identity(nc, identb[:])

        w1sb = wsb[:, :WC_W1].rearrange("p (k d) -> p k d", k=KT)
        wa1sb = wsb[:, o_wa:o_wa + WC_WA].rearrange("p (k d) -> p k d", k=KT)
        w2sb = wsb[:, o_w2:o_w2 + WC_W2].rearrange("p (k d) -> p k d", k=K2)
        b1sb = wsb[:, o_b1:o_b1 + DH]
        disb = wsb[:, o_di:o_di + cfg.sum_ng]
        iosb = wsb[:, o_io:o_io + P * NGM].rearrange("p (d g) -> p d g", d=P)

        half = cfg.hid // 2

        def rng3(t):
            return ((cfg.offs_a(t), cfg.ga[t]),
                    (cfg.offs_b1(t), cfg.gb1[t]),
                    (cfg.offs_b2(t), cfg.gb2[t]))

        def a_tile(t):
            lt = xosb[:, :, t * P:(t + 1) * P]
            ph = psB_p.tile([P, DH], F32, tag="big")
            pa_t = psS_p.tile([P, 256], F32, tag="sm")
            pa = pa_t[:, :2 * HD]
            for k in range(KT):
                nc.tensor.matmul(ph[:], lt[:, k, :], w1sb[:, k, :],
                                 start=(k == 0), stop=(k == KT - 1))
            for k in range(KT):
                nc.tensor.matmul(pa[:], lt[:, k, :], wa1sb[:, k, :],
                                 start=(k == 0), stop=(k == KT - 1))
            hs = hsb_p.tile([P, RW1U], BF16, tag="hsb")
            hsr = hs[:, :DH].rearrange("p (c h) -> p h c", h=HD)
            phr = ph[:].rearrange("p (h c) -> p h c", h=HD)
            nc.vector.tensor_copy(hsr[:, :, :half], phr[:, :, :half])
            nc.scalar.copy(hsr[:, :, half:], phr[:, :, half:])
            nc.scalar.copy(hs[:, DH:DH + HD], pa[:, :HD])
            nc.scalar.copy(ad_bf[:, t, :], pa[:, HD:2 * HD])
            if cfg.collective:
                hdst = hoa if t < KA else hob
                r0 = (t if t < KA else t - KA) * P
            else:
                hdst = haug
                r0 = t * P if t < KA else NA + (t - KA) * P
            nc.sync.dma_start(out=hdst[r0:r0 + P, :RW1U], in_=hs[:])

        def c2_build(t):
            for off, ng in rng3(t):
                if ng == 0:
                    continue
                nc.vector.tensor_tensor(
                    out=c2all[:, :, off:off + ng],
                    in0=disb[:, off:off + ng].unsqueeze(1)
                        .broadcast_to([P, P, ng]),
                    in1=iosb[:, :, :ng],
                    op=mybir.AluOpType.is_equal)

        def ade_pre(t):
            pad_e_t = psS_p.tile([P, 256], F32, tag="sm")
            j = 0
            for off, ng in rng3(t):
                if ng == 0:
                    continue
                e0 = P * off
                nc.sync.dma_start(out=ctall[:, e0:e0 + P * ng],
                                  in_=ct2[:, e0:e0 + P * ng])
                for g in range(ng):
                    nc.tensor.matmul(
                        pad_e_t[:, (j + g) * HD:(j + g + 1) * HD],
                        ctall[:, e0 + g * P:e0 + (g + 1) * P],
                        ad_bf[:, t, :], start=True, stop=True)
                nc.scalar.copy(
                    ade_sb[:, off:off + ng, :],
                    pad_e_t[:, j * HD:(j + ng) * HD]
                    .rearrange("p (g h) -> p g h", h=HD))
                j += ng

        doB = "B" in cfg.phases
        doC = "C" in cfg.phases
        doD = "D" in cfg.phases

        def gather_a(t):
            if not cfg.ga[t]:
                return None
            hga = hga_p.tile([P, max(cfg.ga), cfg.rw1], BF16, tag="hga")
            off = cfg.offs_a(t)
            e0, nidx = P * off, P * cfg.ga[t]
            nc.gpsimd.dma_gather(
                out_ap=hga[:, :cfg.ga[t], :], in_ap=haug[0:NA, :],
                idxs_ap=gisb[:, e0 // 16:(e0 + nidx) // 16],
                num_idxs=nidx, num_idxs_reg=nidx, elem_size=cfg.rw1,
                single_packet=False)
            return hga

        # ---- phase A + AllGathers ----
        ADE_PRE = 4
        NPRE = 4
        pre_a = {}
        for t in range(KA):
            a_tile(t)
            c2_build(t)
            if t < ADE_PRE:
                ade_pre(t)
        if cfg.collective:
            nc.gpsimd.collective_compute(
                "AllGather", mybir.AluOpType.bypass,
                ins=[hoa[:]], outs=[haug[0:NA, :]], replica_groups=grp)
        if doB:
            for t in range(NPRE):
                pre_a[t] = gather_a(t)
        for t in range(KA, cfg.tpc):
            a_tile(t)
            c2_build(t)
        if cfg.collective:
            nc.gpsimd.collective_compute(
                "AllGather", mybir.AluOpType.bypass,
                ins=[hob[:]], outs=[haug[NA:, :]], replica_groups=grp)

        # ---- phase B ----
        def b_front(t):
            parts = []
            if cfg.ga[t]:
                hga = pre_a.pop(t) if t in pre_a else gather_a(t)
                parts.append((cfg.offs_a(t), cfg.ga[t], hga))
            for q, gl, pool, tg in ((1, cfg.gb1, hgb1_p, "hgb1"),
                                    (2, cfg.gb2, hgb2_p, "hgb2")):
                if not gl[t]:
                    continue
                buf = pool.tile([P, max(gl), cfg.rw1], BF16, tag=tg)
                off = cfg.offs_b1(t) if q == 1 else cfg.offs_b2(t)
                e0, nidx = P * off, P * gl[t]
                nc.gpsimd.dma_gather(
                    out_ap=buf[:, :gl[t], :], in_ap=haug[:, :],
                    idxs_ap=gisb[:, e0 // 16:(e0 + nidx) // 16],
                    num_idxs=nidx, num_idxs_reg=nidx, elem_size=cfg.rw1,
                    single_packet=False)
                parts.append((off, gl[t], buf))
            psD_t = psS_p.tile([P, 256], F32, tag="sm")
            psY = psB_p.tile([P, DH], F32, tag="big")
            ngtot = sum(ng for _, ng, _ in parts)
            # batched per engine: DVE adds, ACT prelu+exp, PE den-mm,
            # DVE mults, PE psY-mm
            for off, ng, buf in parts:
                ex = buf[:, :ng, DH:DH + HD]
                nc.vector.tensor_tensor(
                    out=ex, in0=ex, in1=ade_sb[:, off:off + ng, :],
                    op=mybir.AluOpType.add)
            for off, ng, buf in parts:
                ex = buf[:, :ng, DH:DH + HD]
                nc.scalar.activation(ex, ex,
                                     mybir.ActivationFunctionType.Prelu,
                                     alpha=NEG_SLOPE)
                nc.scalar.activation(ex, ex,
                                     mybir.ActivationFunctionType.Exp)
            gdone = 0
            for off, ng, buf in parts:
                hgv = buf[:, :ng, :]
                for g in range(ng):
                    nc.tensor.matmul(psD_t[:, :HD],
                                     c2all[:, :, off + g],
                                     hgv[:, g, DH:DH + HD],
                                     start=(gdone + g == 0),
                                     stop=(gdone + g == ngtot - 1),
                                     skip_group_check=True)
                gdone += ng
            for off, ng, buf in parts:
                hgv = buf[:, :ng, :]
                hgm = hgv[:, :, :DH].rearrange("p g (c h) -> p g c h", h=HD)
                ex = hgv[:, :, DH:DH + HD]
                nc.vector.tensor_tensor(
                    out=hgm, in0=hgm,
                    in1=ex.unsqueeze(2).broadcast_to([P, ng, cfg.hid, HD]),
                    op=mybir.AluOpType.mult)
            gdone = 0
            for off, ng, buf in parts:
                hgv = buf[:, :ng, :]
                for g in range(ng):
                    nc.tensor.matmul(psY[:], c2all[:, :, off + g],
                                     hgv[:, g, :DH],
                                     start=(gdone + g == 0),
                                     stop=(gdone + g == ngtot - 1),
                                     skip_group_check=True)
                gdone += ng
            return psY, psD_t

        def b_back(t, psY, psD):
            den = sm_p.tile([P, HD], F32, tag="den")
            nc.vector.tensor_scalar_max(out=den[:], in0=psD[:, :HD],
                                        scalar1=1e-30)
            rec = sm_p.tile([P, HD], F32, tag="rec")
            nc.vector.reciprocal(rec[:], den[:])
            y = big_p.tile([P, DH], BF16, tag="y")
            nc.vector.tensor_tensor(
                out=y[:].rearrange("p (c h) -> p c h", h=HD),
                in0=psY[:].rearrange("p (c h) -> p c h", h=HD),
                in1=rec[:].unsqueeze(1).broadcast_to([P, cfg.hid, HD]),
                op=mybir.AluOpType.mult)
            nc.vector.tensor_add(y[:], y[:], b1sb[:])
            tneg = big_p.tile([P, DH], BF16, tag="tneg")
            nc.scalar.activation(tneg[:], y[:],
                                 mybir.ActivationFunctionType.Relu, scale=-1.0)
            nc.scalar.activation(y[:], y[:],
                                 mybir.ActivationFunctionType.Relu)
            nc.scalar.activation(tneg[:], tneg[:],
                                 mybir.ActivationFunctionType.Exp, scale=-1.0)
            nc.vector.scalar_tensor_tensor(
                out=y[:], in0=tneg[:], scalar=-1.0, in1=y[:],
                op0=mybir.AluOpType.add, op1=mybir.AluOpType.add)
            if not doC:
                return
            pt = psT_p.tile([P, K2, P], BF16, tag="pt")
            for k in range(K2):
                nc.tensor.transpose(pt[:, k, :], y[:, k * P:(k + 1) * P],
                                    identb[:])
            yT = big_p.tile([P, K2, P], BF16, tag="yT")
            nc.scalar.copy(yT[:], pt[:])
            p2_t = psS_p.tile([P, 256], F32, tag="sm")
            p2 = p2_t[:, :DO + 2]
            for k in range(K2):
                nc.tensor.matmul(p2[:], yT[:, k, :], w2sb[:, k, :],
                                 start=(k == 0), stop=(k == K2 - 1))
            h2sb = out_p.tile([P, DO + 1], BF16, tag="h2sb")
            nc.scalar.copy(h2sb[:], p2[:, :DO + 1])
            nc.scalar.copy(ad2_bf[:, t, :], p2[:, DO + 1:DO + 2])
            if cfg.collective:
                if t < KA:
                    h2dst, r0 = h2oa, t * P
                elif t < KB1:
                    h2dst, r0 = h2ob1, (t - KA) * P
                else:
                    h2dst, r0 = h2ob2, (t - KB1) * P
            else:
                if t < KA:
                    h2dst, r0 = h2tab, t * P
                elif t < KB1:
                    h2dst, r0 = h2tb1, (t - KA) * P
                else:
                    h2dst, r0 = h2tab, NA + (t - KB1) * P
            nc.sync.dma_start(out=h2dst[r0:r0 + P, :DO + 1], in_=h2sb[:])

        def pad2_pre(t):
            pq_t = psS_p.tile([P, 256], F32, tag="sm")
            j = 0
            for off, ng in rng3(t):
                for g in range(ng):
                    nc.tensor.matmul(
                        pq_t[:, j + g:j + g + 1],
                        ctall[:, P * (off + g):P * (off + g + 1)],
                        ad2_bf[:, t, :], start=True, stop=True)
                j += ng
            j = 0
            for off, ng in rng3(t):
                if ng:
                    nc.scalar.copy(ad2e_sb[:, off:off + ng],
                                   pq_t[:, j:j + ng])
                j += ng

        # ---- phase D helpers ----
        def d_gather(ts, q, buf):
            gl = (cfg.ga, cfg.gb1, cfg.gb2)[q]
            ngsum = sum(gl[t] for t in ts)
            if ngsum == 0:
                return
            off0 = (cfg.offs_a(ts[0]), cfg.offs_b1(ts[0]),
                    cfg.offs_b2(ts[0]))[q]
            e0 = P * off0
            nidx = P * ngsum
            src = (h2tab[0:NA, :],
                   h2tb1[:, :] if h2tb1 is not None else None,
                   h2tab[:, :])[q]
            nc.gpsimd.dma_gather(
                out_ap=buf[:, :ngsum, :], in_ap=src,
                idxs_ap=gi2sb[:, e0 // 16:(e0 + nidx) // 16],
                num_idxs=nidx, num_idxs_reg=nidx, elem_size=cfg.rw2,
                single_packet=False)

        def d_chunk(ts, q, buf, psO):
            """ex2 chain + scale + per-tile scatter for one gathered chunk.
            Per-tile psum regions: q==0 standalone; q==1 opens, q==2 closes
            (flags adapt when a tile has no groups in one of the ranges)."""
            gl = (cfg.ga, cfg.gb1, cfg.gb2)[q]
            ngs = [gl[t] for t in ts]
            ngsum = sum(ngs)
            if ngsum == 0:
                return
            off0 = (cfg.offs_a(ts[0]), cfg.offs_b1(ts[0]),
                    cfg.offs_b2(ts[0]))[q]
            hv = buf[:, :ngsum, :]
            ex2 = hv[:, :, DO:DO + 1]
            nc.vector.tensor_tensor(
                out=ex2, in0=ex2,
                in1=ad2e_sb[:, off0:off0 + ngsum].unsqueeze(2),
                op=mybir.AluOpType.add)
            nc.scalar.activation(ex2, ex2,
                                 mybir.ActivationFunctionType.Prelu,
                                 alpha=NEG_SLOPE)
            nc.scalar.activation(ex2, ex2, mybir.ActivationFunctionType.Exp)
            nc.vector.tensor_tensor(
                out=hv[:, :, :DO], in0=hv[:, :, :DO],
                in1=ex2.broadcast_to([P, ngsum, DO]),
                op=mybir.AluOpType.mult)
            co = 0
            for i, t in enumerate(ts):
                off = (cfg.offs_a(t), cfg.offs_b1(t), cfg.offs_b2(t))[q]
                if q == 0:
                    st_t, sp_t = True, True
                else:
                    st_t, sp_t = True, True
                for g in range(ngs[i]):
                    nc.tensor.matmul(
                        psO[:, i * (DO + 1):(i + 1) * (DO + 1)],
                        c2all[:, :, off + g], hv[:, co + g, :DO + 1],
                        start=(st_t and g == 0),
                        stop=(sp_t and g == ngs[i] - 1),
                        skip_group_check=True)
                co += ngs[i]

        def d_finish(ts, psOb):
            n = len(ts)
            t0 = ts[0]
            osb_ch = out_p.tile([P, cfg.tpc // 2, DO], F32, tag="osbch")
            # dense ACT copy of the chunk psums to SBUF, then chunk-wide
            # math (strided SBUF reads are exact; strided PSUM is not)
            stg = sm2_p.tile([P, (cfg.tpc // 2) * (DO + 1)], F32,
                             tag="fstg")
            nc.scalar.copy(stg[:, :n * (DO + 1)], psOb[:, :n * (DO + 1)])
            pr = stg[:, :n * (DO + 1)].rearrange("p (i d) -> p i d",
                                                 d=DO + 1)
            tot = sm2_p.tile([P, cfg.tpc // 2, DO + 1], F32, tag="tot")
            nc.vector.tensor_tensor(out=tot[:, :n, :], in0=pr[:],
                                    in1=osbA[:, t0:t0 + n, :],
                                    op=mybir.AluOpType.add)
            if KB1 > KA:
                nc.vector.tensor_tensor(out=tot[:, :n, :],
                                        in0=tot[:, :n, :],
                                        in1=osbB[:, t0:t0 + n, :],
                                        op=mybir.AluOpType.add)
            den2 = sm2_p.tile([P, cfg.tpc // 2], F32, tag="den2")
            rec2 = sm2_p.tile([P, cfg.tpc // 2], F32, tag="rec2")
            nc.vector.tensor_scalar_max(out=den2[:, :n],
                                        in0=tot[:, :n, DO],
                                        scalar1=1e-30)
            nc.vector.reciprocal(rec2[:, :n], den2[:, :n])
            for i, t in enumerate(ts):
                nc.vector.scalar_tensor_tensor(
                    out=osb_ch[:, i, :], in0=tot[:, i, :DO],
                    scalar=rec2[:, i:i + 1], in1=b2sb[:],
                    op0=mybir.AluOpType.mult, op1=mybir.AluOpType.add)
            nc.sync.dma_start(
                out=out[t0 * P:(t0 + n) * P, :].rearrange(
                    "(i p) d -> p i d", p=P),
                in_=osb_ch[:, :n, :])

        CH = [list(range(0, 5)), list(range(5, cfg.tpc))]
        CH2MAX = max(max(sum(cfg.ga[t] for t in ts) for ts in CH),
                     max(sum(cfg.gb1[t] for t in ts) for ts in CH),
                     max(sum(cfg.gb2[t] for t in ts) for ts in CH), 1)

        def ag_h2(block):
            if not cfg.collective:
                return
            if block == "b1" and KB1 == KA:
                return
            if block == "b2" and cfg.tpc == KB1:
                return
            ins_, outt, lo, hi = {
                "a": (h2oa, h2tab, 0, NA),
                "b1": (h2ob1, h2tb1, 0, NB1),
                "b2": (h2ob2, h2tab, NA,
                       NA + (cfg.tpc - KB1) * P * cfg.n_cores),
            }[block]
            nc.gpsimd.collective_compute(
                "AllGather", mybir.AluOpType.bypass,
                ins=[ins_[:]], outs=[outt[lo:hi, :]], replica_groups=grp)

        if doB:
            state = {}
            dbufs = {}
            for t in range(cfg.tpc):
                if ADE_PRE + t < cfg.tpc:
                    ade_pre(ADE_PRE + t)
                state[t] = b_front(t)
                if t >= 1:
                    b_back(t - 1, *state.pop(t - 1))
                    if doD:
                        pad2_pre(t - 1)
            if doD and doC:
                # AG-a + D-a gathers queue behind the last B gathers
                ag_h2("a")
                for ci, ts in enumerate(CH):
                    buf = hg2_p.tile([P, CH2MAX, cfg.rw2], BF16, tag="hg2")
                    dbufs[("a", ci)] = buf
                    d_gather(ts, 0, buf)
            b_back(cfg.tpc - 1, *state.pop(cfg.tpc - 1))
            if doD:
                pad2_pre(cfg.tpc - 1)

        if doB and doC and doD:
            psOs = {}
            # a-chunk computes (data arrived during B tail) -> osbA;
            # then b1 gathers reuse the a-buffers (read-before-write order)
            for ci, ts in enumerate(CH):
                psOa = psS_p.tile([P, 256], F32, tag="sm")
                if sum(cfg.ga[t] for t in ts):
                    d_chunk(ts, 0, dbufs[("a", ci)], psOa)
                n = len(ts)
                nc.scalar.copy(
                    osbA[:, ts[0]:ts[0] + n, :],
                    psOa[:, :n * (DO + 1)].rearrange(
                        "p (i d) -> p i d", d=DO + 1))
                for t in ts:
                    if cfg.ga[t] == 0:
                        nc.vector.memset(osbA[:, t, :], 0.0)
                if ci == 0:
                    ag_h2("b1")
                buf = hg2_p.tile([P, CH2MAX, cfg.rw2], BF16, tag="hg2")
                dbufs[("b1", ci)] = buf
                d_gather(ts, 1, buf)


            ag_h2("b2")
            osbB = cst.tile([P, cfg.tpc, DO + 1], F32) if KB1 > KA else None
            for ci, ts in enumerate(CH):
                n = len(ts)
                if sum(cfg.gb1[t] for t in ts):
                    psOb = psS_p.tile([P, 256], F32, tag="sm")
                    d_chunk(ts, 1, dbufs[("b1", ci)], psOb)
                    nc.scalar.copy(
                        osbB[:, ts[0]:ts[0] + n, :],
                        psOb[:, :n * (DO + 1)].rearrange(
                            "p (i d) -> p i d", d=DO + 1))
                    for t in ts:
                        if cfg.gb1[t] == 0:
                            nc.vector.memset(osbB[:, t, :], 0.0)
                elif KB1 > KA:
                    nc.vector.memset(osbB[:, ts[0]:ts[0] + n, :], 0.0)
                buf = hg2_p.tile([P, CH2MAX, cfg.rw2], BF16, tag="hg2")
                dbufs[("b2", ci)] = buf
                d_gather(ts, 2, buf)
            import os as _osd
            if _osd.environ.get("DBG_AB"):
                dbgA = nc.dram_tensor("dbgA", [P, cfg.tpc * (DO + 1)], F32,
                                      kind="ExternalOutput")
                nc.sync.dma_start(out=dbgA[:],
                                  in_=osbA[:].rearrange("p a b -> p (a b)"))
                dbgB = nc.dram_tensor("dbgB", [P, cfg.tpc * (DO + 1)], F32,
                                      kind="ExternalOutput")
                nc.sync.dma_start(out=dbgB[:],
                                  in_=osbB[:].rearrange("p a b -> p (a b)"))
            for ci, ts in enumerate(CH):
                psOb2 = psS_p.tile([P, 256], F32, tag="sm")
                psOs[ci] = psOb2
                d_chunk(ts, 2, dbufs[("b2", ci)], psOb2)
                d_finish(ts, psOs[ci])

    nc.compile()
    return nc


def default_cfg() -> Cfg:
    return Cfg()


def run(inputs: dict, cfg: Cfg | None = None, **run_kwargs):
    cfg = cfg or default_cfg()
    in_maps, node_of_slot = preprocess(cfg, **inputs)
    nc = build_program(cfg)
    res = run_bass_kernel_spmd(nc, in_maps, list(range(cfg.n_cores)),
                               **run_kwargs)
    outs = np.concatenate([res.results[c]["out"] for c in range(cfg.n_cores)],
                          axis=0)
    full = np.zeros((cfg.n_nodes, cfg.d_out), np.float32)
    flat = node_of_slot.reshape(-1)
    real = flat >= 0
    full[flat[real]] = outs[real]
    return full, res


def kernel(**inputs) -> np.ndarray:
    out, _ = run(inputs)
    return out
